# revision 38
# baseline (speedup 1.0000x reference)
"""Trainium2 Bass kernel for nn_MelGroupConv (16 mel-subband group convs).

Math: the reference applies, per subband group g_i, two chained conv2d
(3,1) layers (g_i -> 64 -> 8 channels) along time with zero padding and
NO nonlinearity between them, so the composition collapses to a single
5-tap conv per group:

    z[o, t] = sum_{c, j} Weff[o, c, j] x[c, t + j - 2]   (x zero-padded by 2)

with two exact boundary corrections (the inner conv's zero padding acts
on h, not x): at t=0 subtract (w2[:,:,0] @ w1[:,:,2]) x[:, 0], at t=T-1
subtract (w2[:,:,2] @ w1[:,:,0]) x[:, T-1]. Biases pass through linearly
and reduce to a per-(o, t-boundary) additive field applied host-side
(identically zero for this problem's inputs). Weff / corrective weights
are precomputed on host in float64.

Distribution: data-parallel, 4 of the 32 batches per core.

Device compute, per (batch, 500-column time chunk):
  - block 0: channels 0..117  -> psum0 (96 outputs, groups 0-11), 5
    tap-shifted fp32r matmuls accumulating in PSUM;
  - block 1: channels 118..245 -> psum1 (32 outputs, groups 12-15), 5
    tap matmuls;
  - block 2: channels 246..256 tap-FOLDED host-side onto 55 partitions
    (5 shifted copies of 11 channels) -> ONE matmul into psum1;
  - 3 corrective N=2 matmuls fix the boundary columns (staging columns
    baked into the x layout host-side);
  - DVE/ACT copy psum -> SBUF; DMAs write DRAM per chunk.
11 main matmuls per (batch, chunk) x 8 = 88 N=500 matmuls per core.

fp32r (fp32 rounded to 12 mantissa bits, done host-side so any engine
may DMA it) runs the PE at 1 cycle/row (4x full fp32) and gives ~1.6e-4
max relative error. ISA constraints honored: fp32r requires col_grp=0xf
(so every matmul's PSUM target starts at partition 0 -> two PSUM tiles
per chunk), even N, 8B-aligned PSUM dst.

Scheduling (engines balanced, cost-model-driven):
  - input/weight DMAs all on SP so the shared DMA-engine pool drains
    them in consumption order; transfers are the serialized resource, so
    first-needed tiles go first and large x transfers are split in
    halves for just-in-time compute starts;
  - 6 warmup matmuls on zeroed scratch keep the PE (and the HAM clock
    on real silicon) busy while the first transfers land; their PSUM
    output is overwritten by the first start=True matmul;
  - psum0 copies on DVE, psum1 copies on ACT, y0 output DMAs issued on
    gpsimd/SWDGE, y1 on ACT/HWDGE, spreading issue costs (~0.6-1us per
    dma_start) across all four non-PE engines;
  - per-batch epilogues right after each batch's last block so copies
    and output DMAs overlap remaining matmuls.
"""
import numpy as np

import concourse.bass as bass
import concourse.mybir as mybir
import concourse.tile as tile
from concourse.bass_utils import run_bass_kernel_spmd

GROUPS = (6, 6, 6, 6, 6, 7, 8, 9, 12, 14, 17, 21, 25, 31, 38, 45)
N_GROUPS = 16
B, F, T = 32, 257, 1000
N_CORES = 8
BL = B // N_CORES  # batches per core
F32 = mybir.dt.float32
F32R = mybir.dt.float32r

# Channel K-blocks: (ch_start, K, weight_col_offset_within_tap, M, psum_idx)
# psum 0 holds output rows 0..95 (groups 0-11); psum 1 rows 96..127
# (groups 12-15). Block 1 carries channels 118..245 (groups 12-14 plus
# the first 34 channels of group 15); block 2 the remaining 11 channels
# of group 15, accumulated into psum 1.
BLOCKS = ((0, 118, 0, 96, 0), (118, 128, 96, 32, 1), (246, 11, 128, 32, 1))
# Block 2 is tap-folded: its 11 channels x 5 taps live on 55 partitions of a
# dedicated x tile (one matmul per (batch, chunk) instead of five). Folded
# row order puts the j=2 (centre-tap, unshifted) block first so the N=2
# corrective matmul's rhs starts at partition 0.
FOLD_JORDER = (2, 0, 1, 3, 4)  # folded row block j for rows [k*11:(k+1)*11]
NF = 55
TAP_STRIDE = 128  # 96 + 32 weight columns per tap slice (blocks 0 and 1)
CORR_STRIDE = 160  # corrective slices also carry block 2 (96 + 32 + 32)
NCHUNK = 2
CHUNK = T // NCHUNK  # 500
# Per-(block, chunk) x tile: [K, BL * XTW]; batch b occupies columns
# b*XTW .. (b+1)*XTW. Within a batch segment (width 508):
#   cols 0,1 : corrective staging ([x(0), 0] for chunk 0; [0, x(T-1)] for 1)
#   cols 2..507 : x[t'] for t' = c*CHUNK-2 .. c*CHUNK+503, zero outside [0,T)
# Tap-j matmul rhs window = segment cols 2+j .. 2+j+CHUNK.
XTW = CHUNK + 8


def _fix_multiwaits(nc):
    """This walrus build rejects >1 sync-wait per engine instruction.
    Hoist extras onto dedicated single-wait nops just before the
    instruction in its basic block (same engine: identical blocking)."""
    for f in nc.m.functions:
        for bb in f.blocks:
            il = bb.instructions
            i = 0
            while i < len(il):
                inst = il[i]
                si = inst.sync_info
                if si is not None and si.on_wait is not None and len(si.on_wait) > 1:
                    waits = list(si.on_wait)
                    while len(si.on_wait) > 1:
                        si.on_wait.pop(0)
                    for w in waits[:-1]:
                        nop = mybir.InstNoOp(
                            name=nc.get_next_instruction_name(),
                            bass_nofuse=True,
                            engine=inst.engine,
                            sync_info=mybir.SyncInfo(on_wait=[w], on_update=[]),
                        )
                        il.insert(i, nop)
                        i += 1
                i += 1


def _build_weights(w1, b1, w2, b2):
    """Host-side float64 weight preprocessing.

    Returns (WT [128, NW] float32 packed lhsT slices,
             bias_field [128, T] float64)."""
    starts = np.concatenate([[0], np.cumsum(GROUPS)])
    W5 = np.zeros((F, 128, 5))  # [channel, out_col, tap]
    A0 = np.zeros((F, 128))
    A9 = np.zeros((F, 128))
    bias_field = np.zeros((128, T))
    b1 = np.asarray(b1, dtype=np.float64)
    b2 = np.asarray(b2, dtype=np.float64)
    for i, g in enumerate(GROUPS):
        s = starts[i]
        w1g = np.asarray(w1[i], dtype=np.float64)[:, :, :, 0]  # [64, g, 3]
        w2g = np.asarray(w2[i], dtype=np.float64)[:, :, :, 0]  # [8, 64, 3]
        for k2 in range(3):
            m2 = w2g[:, :, k2]
            for k1 in range(3):
                W5[s : s + g, i * 8 : i * 8 + 8, k1 + k2] += (m2 @ w1g[:, :, k1]).T
        A0[s : s + g, i * 8 : i * 8 + 8] = (w2g[:, :, 0] @ w1g[:, :, 2]).T
        A9[s : s + g, i * 8 : i * 8 + 8] = (w2g[:, :, 2] @ w1g[:, :, 0]).T
        bias_t = np.einsum("ohk,h->ok", w2g, b1[i])  # [8, 3]
        bias_int = bias_t.sum(axis=1) + b2[i]
        rows = slice(i * 8, i * 8 + 8)
        bias_field[rows, :] = bias_int[:, None]
        bias_field[rows, 0] -= bias_t[:, 0]
        bias_field[rows, T - 1] -= bias_t[:, 2]

    WTAP = np.zeros((128, 5 * TAP_STRIDE), dtype=np.float64)
    WCOR = np.zeros((128, 2 * CORR_STRIDE), dtype=np.float64)

    def pack(dst, dst_off, src, include_blk2=True):  # src: [F, 128]
        for c0, K, wc, M, pi in BLOCKS:
            if not include_blk2 and wc >= TAP_STRIDE:
                continue
            col0 = 96 if pi else 0
            dst[0:K, dst_off + wc : dst_off + wc + M] = src[
                c0 : c0 + K, col0 : col0 + M
            ]

    for j in range(5):
        pack(WTAP, j * TAP_STRIDE, W5[:, :, j], include_blk2=False)
    pack(WCOR, 0, -A0)
    pack(WCOR, CORR_STRIDE, -A9)
    WBLK2 = np.zeros((NF, 32), dtype=np.float64)
    for k, j in enumerate(FOLD_JORDER):
        WBLK2[k * 11 : (k + 1) * 11, :] = W5[246:257, 96:128, j]
    return (
        WTAP.astype(np.float32),
        WCOR.astype(np.float32),
        WBLK2.astype(np.float32),
        bias_field,
    )


def _build_nc():
    nc = bass.Bass("TRN2", target_bir_lowering=False, debug=False, num_devices=N_CORES)
    # x rows = channel (block-ordered = natural order); cols = (chunk, batch, seg)
    x = nc.dram_tensor(
        "x", [246 + NF, NCHUNK * BL * XTW], F32R, kind="ExternalInput"
    ).ap()
    wblk2_d = nc.dram_tensor("wblk2", [NF, 32], F32R, kind="ExternalInput").ap()
    wtap_d = nc.dram_tensor(
        "wtap", [128, 5 * TAP_STRIDE], F32R, kind="ExternalInput"
    ).ap()
    wcor_d = nc.dram_tensor(
        "wcor", [128, 2 * CORR_STRIDE], F32R, kind="ExternalInput"
    ).ap()
    y = nc.dram_tensor("y", [BL * 128, T], F32, kind="ExternalOutput").ap()
    with tile.TileContext(nc) as tc:
        with (
            tc.tile_pool(name="sb", bufs=1) as sb,
            tc.tile_pool(name="ob", bufs=1) as ob,
            tc.tile_pool(name="ps", bufs=1, space="PSUM") as ps,
        ):
            wblk2 = sb.tile([NF, 32], F32R, name="wblk2", tag="wblk2")
            wtap = sb.tile([128, 5 * TAP_STRIDE], F32R, name="wtap", tag="wtap")
            xts = {}
            cw = BL * XTW
            for c in range(NCHUNK):
                for c0 in (246, 0, 118):
                    xts[(c, c0)] = sb.tile(
                        [128, BL * XTW], F32R, name=f"x{c}_{c0}", tag=f"x{c}_{c0}"
                    )

            def xdma(eng, c, c0, K):
                eng.dma_start(
                    xts[(c, c0)][0:K, :], x[c0 : c0 + K, c * cw : (c + 1) * cw]
                )

            # PE warmup: matmuls on zeroed scratch keep the PE busy (and the
            # HAM clock warm) while the first input DMAs land. Results go to
            # the first PSUM tile and are overwritten by the real start=True
            # matmuls later.
            scratch = sb.tile([128, 512], F32, name="scratch", tag="scratch")
            nc.gpsimd.memset(scratch[:, :], 0.0)

            # Issue order tuned for earliest PE start. All input DMAs go on
            # SP (HWDGE) so the shared DMA-engine pool drains them in exactly
            # this order; transfers serialize on that pool, so first-needed
            # and small DMAs go first. wcor is small and needed by the first
            # chunk-0 epilogue (~12us in) - it rides right after wtap.
            wcor = sb.tile([128, 2 * CORR_STRIDE], F32R, name="wcor", tag="wcor")
            t00 = xts[(0, 0)]
            nc.sync.dma_start(wblk2[:], wblk2_d[:])
            xdma(nc.sync, 0, 246, NF)
            nc.sync.dma_start(
                wtap[:, 0:TAP_STRIDE], wtap_d[:, 0:TAP_STRIDE]
            )
            nc.sync.dma_start(t00[0:118, 0 : 2 * XTW], x[0:118, 0 : 2 * XTW])
            nc.sync.dma_start(
                wtap[:, TAP_STRIDE:], wtap_d[:, TAP_STRIDE:]
            )
            nc.sync.dma_start(t00[0:118, 2 * XTW : cw], x[0:118, 2 * XTW : cw])
            nc.sync.dma_start(wcor[:], wcor_d[:])

            def xdma_halves(c, c0, K):
                t = xts[(c, c0)]
                nc.sync.dma_start(
                    t[0:K, 0 : 2 * XTW], x[c0 : c0 + K, c * cw : c * cw + 2 * XTW]
                )
                nc.sync.dma_start(
                    t[0:K, 2 * XTW : cw],
                    x[c0 : c0 + K, c * cw + 2 * XTW : (c + 1) * cw],
                )

            xdma_halves(0, 118, 128)
            xdma(nc.sync, 1, 246, NF)
            t10 = xts[(1, 0)]
            ccw = cw
            nc.sync.dma_start(
                t10[0:118, 0 : 2 * XTW], x[0:118, ccw : ccw + 2 * XTW]
            )
            nc.sync.dma_start(
                xts[(1, 118)][0:128, 0 : 2 * XTW],
                x[118:246, ccw : ccw + 2 * XTW],
            )
            nc.sync.dma_start(
                t10[0:118, 2 * XTW : cw], x[0:118, ccw + 2 * XTW : 2 * ccw]
            )
            nc.sync.dma_start(
                xts[(1, 118)][0:128, 2 * XTW : cw],
                x[118:246, ccw + 2 * XTW : 2 * ccw],
            )

            o0s, o1s = [], []
            for b in range(BL):
                o0s.append(ob.tile([96, T], F32, name=f"o0_{b}", tag=f"o0_{b}"))
                o1s.append(ob.tile([32, T], F32, name=f"o1_{b}", tag=f"o1_{b}"))

            def mm_block(bi, c, b, psts):
                c0, K, wc, M, pi = BLOCKS[bi]
                if bi == 2:
                    xt = xts[(c, 246)]
                    nc.tensor.matmul(
                        psts[b][1][:, :],
                        wblk2[0:NF, 0:32],
                        xt[0:NF, b * XTW + 2 : b * XTW + 2 + CHUNK],
                        start=True,
                        stop=False,
                    )
                    return
                xt = xts[(c, c0)]
                for j in range(5):
                    wap = wtap[0:K, j * TAP_STRIDE + wc : j * TAP_STRIDE + wc + M]
                    nc.tensor.matmul(
                        psts[b][pi][:, :],
                        wap,
                        xt[0:K, b * XTW + 2 + j : b * XTW + 2 + j + CHUNK],
                        start=(j == 0 and bi != 1),
                        stop=False,
                    )

            def epilogue(c, b, psts):
                corr_off = 0 if c == 0 else CORR_STRIDE
                pcol = 0 if c == 0 else CHUNK - 2
                for bi, (c0, K, wc, M, pi) in enumerate(BLOCKS):
                    xkey = 246 if bi == 2 else c0
                    nc.tensor.matmul(
                        psts[b][pi][:, pcol : pcol + 2],
                        wcor[0:K, corr_off + wc : corr_off + wc + M],
                        xts[(c, xkey)][0:K, b * XTW : b * XTW + 2],
                        start=False,
                        stop=(bi != 1),
                        skip_group_check=True,
                    )
                nc.vector.tensor_copy(
                    o0s[b][:, c * CHUNK : (c + 1) * CHUNK], psts[b][0][:, :]
                )
                nc.scalar.mul(
                    o1s[b][:, c * CHUNK : (c + 1) * CHUNK], psts[b][1][:, :], 1.0
                )
                cs = slice(c * CHUNK, (c + 1) * CHUNK)
                y0_eng = nc.sync if (c == NCHUNK - 1 and b == BL - 1) else nc.gpsimd
                y0_eng.dma_start(y[b * 128 : b * 128 + 96, cs], o0s[b][:, cs])
                nc.scalar.dma_start(
                    y[b * 128 + 96 : b * 128 + 128, cs], o1s[b][:, cs]
                )

            def psum_tiles(c):
                p0s = [
                    ps.tile([96, CHUNK], F32, name=f"p0_{c}_{b}", tag=f"p0_{b}")
                    for b in range(BL)
                ]
                p1s = [
                    ps.tile([32, CHUNK], F32, name=f"p1_{c}_{b}", tag=f"p1_{b}")
                    for b in range(BL)
                ]
                return [(p0s[b], p1s[b]) for b in range(BL)]

            # chunk 0: block-major (matmuls start as soon as each block's
            # DMA lands); each batch's epilogue follows its last block
            psts0 = psum_tiles(0)
            for k in range(6):
                nc.tensor.matmul(
                    psts0[0][0][:, 0:128],
                    scratch[0:128, 0:96],
                    scratch[0:128, 128:256],
                    start=True,
                    stop=(k == 5),
                    skip_group_check=True,
                )
            for bi in (2, 0):
                for b in range(BL):
                    mm_block(bi, 0, b, psts0)
            for b in range(BL):
                mm_block(1, 0, b, psts0)
                epilogue(0, b, psts0)
            # chunk 1: block-major as well (the chunk-1 x transfers land
            # progressively; block-major consumes them in arrival order),
            # with each batch's epilogue right after its last block
            psts1 = psum_tiles(1)
            for bi in (2, 0):
                for b in range(BL):
                    mm_block(bi, 1, b, psts1)
            for b in range(BL):
                mm_block(1, 1, b, psts1)
                epilogue(1, b, psts1)
    _fix_multiwaits(nc)
    return nc


_NC_CACHE = None


def _get_nc():
    global _NC_CACHE
    if _NC_CACHE is None:
        _NC_CACHE = _build_nc()
    return _NC_CACHE


def _round_fp32r(a):
    """Round float32 to fp32r (12 explicit mantissa bits, round-to-nearest;
    matches the PE's reduced-precision operand format)."""
    bits = np.ascontiguousarray(a, dtype=np.float32).view(np.uint32)
    low = bits & np.uint32(0xFFF)
    base = bits & np.uint32(0xFFFFF000)
    up = (low > 0x800) | (
        (low == 0x800) & (((bits >> np.uint32(12)) & np.uint32(1)) == 1)
    )
    return (base + (up.astype(np.uint32) << np.uint32(12))).view(np.float32)


def _prep_inputs(x, w1, b1, w2, b2):
    x = np.asarray(x)
    WTAP, WCOR, WBLK2, bias_field = _build_weights(w1, b1, w2, b2)
    xd = x[:, :, :, 0]  # [B, F, T]
    # [B, FR, NCHUNK, XTW] widened per-chunk layout (see XTW comment).
    # Rows 0..245: channels 0..245 (blocks 0 and 1). Rows 246..300: block-2
    # tap-folded rows — row 246 + k*11 + ch holds channel (246+ch) shifted by
    # tap j=FOLD_JORDER[k]: col m = x[ch, c*CHUNK + m + j - 4].
    FR = 246 + NF
    xw = np.zeros((B, FR, NCHUNK, XTW), dtype=np.float32)
    for c in range(NCHUNK):
        t_lo = c * CHUNK - 2  # t' at col 2
        lo = max(t_lo, 0)
        hi = min(c * CHUNK + CHUNK + 2, T)
        xw[:, 0:246, c, 2 + (lo - t_lo) : 2 + (hi - t_lo)] = xd[:, 0:246, lo:hi]
        for k, j in enumerate(FOLD_JORDER):
            rows = slice(246 + k * 11, 246 + (k + 1) * 11)
            f_lo = c * CHUNK + j - 4  # value at col 0
            lo2 = max(f_lo, 0)
            hi2 = min(f_lo + XTW, T)
            if hi2 > lo2:
                xw[:, rows, c, lo2 - f_lo : hi2 - f_lo] = xd[:, 246:257, lo2:hi2]
    xw[:, 0:246, 0, 0] = xd[:, 0:246, 0]  # corrective staging [x(0), 0]
    xw[:, 0:246, 1, 1] = xd[:, 0:246, T - 1]  # corrective staging [0, x(T-1)]
    # block-2 corrective staging lives on the folded j=2 rows (246..256).
    # The folded data fill can reach cols 0/1 (f_lo >= 0 for chunk 1's
    # j>=2 rows) — clear them first; only rows 246..256 are ever read there.
    xw[:, 246:, :, 0:2] = 0.0
    xw[:, 246:257, 0, 0] = xd[:, 246:257, 0]
    xw[:, 246:257, 1, 1] = xd[:, 246:257, T - 1]
    # device layout per core: [FR, NCHUNK * BL * XTW], col = ((c*BL)+b)*XTW + s
    xs = (
        xw.reshape(N_CORES, BL, FR, NCHUNK, XTW)
        .transpose(0, 2, 3, 1, 4)
        .reshape(N_CORES, FR, NCHUNK * BL * XTW)
    )
    xs = _round_fp32r(np.ascontiguousarray(xs))
    WTAP = _round_fp32r(WTAP)
    WCOR = _round_fp32r(WCOR)
    WBLK2 = _round_fp32r(WBLK2)
    in_maps = [
        {"x": xs[i], "wtap": WTAP, "wcor": WCOR, "wblk2": WBLK2}
        for i in range(N_CORES)
    ]
    return in_maps, bias_field


def kernel(x, w1, b1, w2, b2):
    in_maps, bias_field = _prep_inputs(x, w1, b1, w2, b2)
    res = run_bass_kernel_spmd(_get_nc(), in_maps, core_ids=list(range(N_CORES)))
    out = np.empty((B, 128, T), dtype=np.float64)
    for i in range(N_CORES):
        out[i * BL : (i + 1) * BL] = (
            res.results[i]["y"].reshape(BL, 128, T).astype(np.float64)
        )
    out += bias_field[None, :, :]
    return out.astype(np.float32).reshape(B, N_GROUPS, 8, T, 1)


# revision 41
# speedup vs baseline: 1.0102x; 1.0102x over previous
"""Trainium2 Bass kernel for nn_MelGroupConv (16 mel-subband group convs).

Math: the reference applies, per subband group g_i, two chained conv2d
(3,1) layers (g_i -> 64 -> 8 channels) along time with zero padding and
NO nonlinearity between them, so the composition collapses to a single
5-tap conv per group:

    z[o, t] = sum_{c, j} Weff[o, c, j] x[c, t + j - 2]   (x zero-padded by 2)

with two exact boundary corrections (the inner conv's zero padding acts
on h, not x): at t=0 subtract (w2[:,:,0] @ w1[:,:,2]) x[:, 0], at t=T-1
subtract (w2[:,:,2] @ w1[:,:,0]) x[:, T-1]. Biases pass through linearly
and reduce to a per-(o, t-boundary) additive field applied host-side
(identically zero for this problem's inputs). Weff / corrective weights
are precomputed on host in float64.

Distribution: data-parallel, 4 of the 32 batches per core.

Device compute, per (batch, 500-column time chunk):
  - block 0: channels 0..117  -> psum0 (96 outputs, groups 0-11), 5
    tap-shifted fp32r matmuls accumulating in PSUM;
  - block 1: channels 118..245 -> psum1 (32 outputs, groups 12-15), 5
    tap matmuls;
  - block 2: channels 246..256 tap-FOLDED host-side onto 55 partitions
    (5 shifted copies of 11 channels) -> ONE matmul into psum1;
  - 3 corrective N=2 matmuls fix the boundary columns (staging columns
    baked into the x layout host-side);
  - DVE/ACT copy psum -> SBUF; DMAs write DRAM per chunk.
11 main matmuls per (batch, chunk) x 8 = 88 N=500 matmuls per core.

fp32r (fp32 rounded to 12 mantissa bits, done host-side so any engine
may DMA it) runs the PE at 1 cycle/row (4x full fp32) and gives ~1.6e-4
max relative error. ISA constraints honored: fp32r requires col_grp=0xf
(so every matmul's PSUM target starts at partition 0 -> two PSUM tiles
per chunk), even N, 8B-aligned PSUM dst.

Scheduling (engines balanced, cost-model-driven):
  - input/weight DMAs all on SP so the shared DMA-engine pool drains
    them in consumption order; transfers are the serialized resource, so
    first-needed tiles go first and large x transfers are split in
    halves for just-in-time compute starts;
  - 6 warmup matmuls on zeroed scratch keep the PE (and the HAM clock
    on real silicon) busy while the first transfers land; their PSUM
    output is overwritten by the first start=True matmul;
  - psum0 copies on DVE, psum1 copies on ACT, y0 output DMAs issued on
    gpsimd/SWDGE, y1 on ACT/HWDGE, spreading issue costs (~0.6-1us per
    dma_start) across all four non-PE engines;
  - per-batch epilogues right after each batch's last block so copies
    and output DMAs overlap remaining matmuls.
"""
import numpy as np

import concourse.bass as bass
import concourse.mybir as mybir
import concourse.tile as tile
from concourse.bass_utils import run_bass_kernel_spmd

GROUPS = (6, 6, 6, 6, 6, 7, 8, 9, 12, 14, 17, 21, 25, 31, 38, 45)
N_GROUPS = 16
B, F, T = 32, 257, 1000
N_CORES = 8
BL = B // N_CORES  # batches per core
F32 = mybir.dt.float32
F32R = mybir.dt.float32r

# Channel K-blocks: (ch_start, K, weight_col_offset_within_tap, M, psum_idx)
# psum 0 holds output rows 0..95 (groups 0-11); psum 1 rows 96..127
# (groups 12-15). Block 1 carries channels 118..245 (groups 12-14 plus
# the first 34 channels of group 15); block 2 the remaining 11 channels
# of group 15, accumulated into psum 1.
BLOCKS = ((0, 118, 0, 96, 0), (118, 128, 96, 32, 1), (246, 11, 128, 32, 1))
# Block 2 is tap-folded: its 11 channels x 5 taps live on 55 partitions of a
# dedicated x tile (one matmul per (batch, chunk) instead of five). Folded
# row order puts the j=2 (centre-tap, unshifted) block first so the N=2
# corrective matmul's rhs starts at partition 0.
FOLD_JORDER = (2, 0, 1, 3, 4)  # folded row block j for rows [k*11:(k+1)*11]
NF = 55
TAP_STRIDE = 128  # 96 + 32 weight columns per tap slice (blocks 0 and 1)
CORR_STRIDE = 160  # corrective slices also carry block 2 (96 + 32 + 32)
NCHUNK = 2
CHUNK = T // NCHUNK  # 500
# Per-(block, chunk) x tile: [K, BL * XTW]; batch b occupies columns
# b*XTW .. (b+1)*XTW. Within a batch segment (width 508):
#   cols 0,1 : corrective staging ([x(0), 0] for chunk 0; [0, x(T-1)] for 1)
#   cols 2..507 : x[t'] for t' = c*CHUNK-2 .. c*CHUNK+503, zero outside [0,T)
# Tap-j matmul rhs window = segment cols 2+j .. 2+j+CHUNK.
XTW = CHUNK + 8


def _fix_multiwaits(nc):
    """This walrus build rejects >1 sync-wait per engine instruction.
    Hoist extras onto dedicated single-wait nops just before the
    instruction in its basic block (same engine: identical blocking)."""
    for f in nc.m.functions:
        for bb in f.blocks:
            il = bb.instructions
            i = 0
            while i < len(il):
                inst = il[i]
                si = inst.sync_info
                if si is not None and si.on_wait is not None and len(si.on_wait) > 1:
                    waits = list(si.on_wait)
                    while len(si.on_wait) > 1:
                        si.on_wait.pop(0)
                    for w in waits[:-1]:
                        nop = mybir.InstNoOp(
                            name=nc.get_next_instruction_name(),
                            bass_nofuse=True,
                            engine=inst.engine,
                            sync_info=mybir.SyncInfo(on_wait=[w], on_update=[]),
                        )
                        il.insert(i, nop)
                        i += 1
                i += 1


def _build_weights(w1, b1, w2, b2):
    """Host-side float64 weight preprocessing.

    Returns (WT [128, NW] float32 packed lhsT slices,
             bias_field [128, T] float64)."""
    starts = np.concatenate([[0], np.cumsum(GROUPS)])
    W5 = np.zeros((F, 128, 5))  # [channel, out_col, tap]
    A0 = np.zeros((F, 128))
    A9 = np.zeros((F, 128))
    bias_field = np.zeros((128, T))
    b1 = np.asarray(b1, dtype=np.float64)
    b2 = np.asarray(b2, dtype=np.float64)
    for i, g in enumerate(GROUPS):
        s = starts[i]
        w1g = np.asarray(w1[i], dtype=np.float64)[:, :, :, 0]  # [64, g, 3]
        w2g = np.asarray(w2[i], dtype=np.float64)[:, :, :, 0]  # [8, 64, 3]
        for k2 in range(3):
            m2 = w2g[:, :, k2]
            for k1 in range(3):
                W5[s : s + g, i * 8 : i * 8 + 8, k1 + k2] += (m2 @ w1g[:, :, k1]).T
        A0[s : s + g, i * 8 : i * 8 + 8] = (w2g[:, :, 0] @ w1g[:, :, 2]).T
        A9[s : s + g, i * 8 : i * 8 + 8] = (w2g[:, :, 2] @ w1g[:, :, 0]).T
        bias_t = np.einsum("ohk,h->ok", w2g, b1[i])  # [8, 3]
        bias_int = bias_t.sum(axis=1) + b2[i]
        rows = slice(i * 8, i * 8 + 8)
        bias_field[rows, :] = bias_int[:, None]
        bias_field[rows, 0] -= bias_t[:, 0]
        bias_field[rows, T - 1] -= bias_t[:, 2]

    WTAP = np.zeros((128, 5 * TAP_STRIDE), dtype=np.float64)
    WCOR = np.zeros((128, 2 * CORR_STRIDE), dtype=np.float64)

    def pack(dst, dst_off, src, include_blk2=True):  # src: [F, 128]
        for c0, K, wc, M, pi in BLOCKS:
            if not include_blk2 and wc >= TAP_STRIDE:
                continue
            col0 = 96 if pi else 0
            dst[0:K, dst_off + wc : dst_off + wc + M] = src[
                c0 : c0 + K, col0 : col0 + M
            ]

    for j in range(5):
        pack(WTAP, j * TAP_STRIDE, W5[:, :, j], include_blk2=False)
    pack(WCOR, 0, -A0)
    pack(WCOR, CORR_STRIDE, -A9)
    WBLK2 = np.zeros((NF, 32), dtype=np.float64)
    for k, j in enumerate(FOLD_JORDER):
        WBLK2[k * 11 : (k + 1) * 11, :] = W5[246:257, 96:128, j]
    return (
        WTAP.astype(np.float32),
        WCOR.astype(np.float32),
        WBLK2.astype(np.float32),
        bias_field,
    )


def _build_nc():
    nc = bass.Bass("TRN2", target_bir_lowering=False, debug=False, num_devices=N_CORES)
    # x rows = channel (block-ordered = natural order); cols = (chunk, batch, seg)
    x = nc.dram_tensor(
        "x", [246 + NF, NCHUNK * BL * XTW], F32R, kind="ExternalInput"
    ).ap()
    wblk2_d = nc.dram_tensor("wblk2", [NF, 32], F32R, kind="ExternalInput").ap()
    wtap_d = nc.dram_tensor(
        "wtap", [128, 5 * TAP_STRIDE], F32R, kind="ExternalInput"
    ).ap()
    wcor_d = nc.dram_tensor(
        "wcor", [128, 2 * CORR_STRIDE], F32R, kind="ExternalInput"
    ).ap()
    y = nc.dram_tensor("y", [BL * 128, T], F32, kind="ExternalOutput").ap()
    with tile.TileContext(nc) as tc:
        with (
            tc.tile_pool(name="sb", bufs=1) as sb,
            tc.tile_pool(name="ob", bufs=1) as ob,
            tc.tile_pool(name="ps", bufs=1, space="PSUM") as ps,
        ):
            wblk2 = sb.tile([NF, 32], F32R, name="wblk2", tag="wblk2")
            wtap = sb.tile([128, 5 * TAP_STRIDE], F32R, name="wtap", tag="wtap")
            xts = {}
            cw = BL * XTW
            for c in range(NCHUNK):
                for c0 in (246, 0, 118):
                    xts[(c, c0)] = sb.tile(
                        [128, BL * XTW], F32R, name=f"x{c}_{c0}", tag=f"x{c}_{c0}"
                    )

            def xdma(eng, c, c0, K):
                eng.dma_start(
                    xts[(c, c0)][0:K, :], x[c0 : c0 + K, c * cw : (c + 1) * cw]
                )

            # PE warmup: matmuls on zeroed scratch keep the PE busy (and the
            # HAM clock warm) while the first input DMAs land. Results go to
            # the first PSUM tile and are overwritten by the real start=True
            # matmuls later.
            scratch = sb.tile([128, 512], F32, name="scratch", tag="scratch")
            nc.gpsimd.memset(scratch[:, :], 0.0)

            # Issue order tuned for earliest PE start. All input DMAs go on
            # SP (HWDGE) so the shared DMA-engine pool drains them in exactly
            # this order; transfers serialize on that pool, so first-needed
            # and small DMAs go first. wcor is small and needed by the first
            # chunk-0 epilogue (~12us in) - it rides right after wtap.
            wcor = sb.tile([128, 2 * CORR_STRIDE], F32R, name="wcor", tag="wcor")
            t00 = xts[(0, 0)]
            nc.sync.dma_start(wblk2[:], wblk2_d[:])
            xdma(nc.sync, 0, 246, NF)
            nc.sync.dma_start(
                wtap[:, 0:TAP_STRIDE], wtap_d[:, 0:TAP_STRIDE]
            )
            nc.sync.dma_start(
                wtap[:, TAP_STRIDE:], wtap_d[:, TAP_STRIDE:]
            )
            nc.sync.dma_start(t00[0:118, 0:XTW], x[0:118, 0:XTW])
            nc.sync.dma_start(t00[0:118, XTW : 2 * XTW], x[0:118, XTW : 2 * XTW])
            nc.sync.dma_start(t00[0:118, 2 * XTW : cw], x[0:118, 2 * XTW : cw])
            nc.sync.dma_start(wcor[:], wcor_d[:])

            def xdma_halves(c, c0, K):
                t = xts[(c, c0)]
                nc.sync.dma_start(
                    t[0:K, 0 : 2 * XTW], x[c0 : c0 + K, c * cw : c * cw + 2 * XTW]
                )
                nc.sync.dma_start(
                    t[0:K, 2 * XTW : cw],
                    x[c0 : c0 + K, c * cw + 2 * XTW : (c + 1) * cw],
                )

            xdma_halves(0, 118, 128)
            xdma(nc.sync, 1, 246, NF)
            t10 = xts[(1, 0)]
            ccw = cw
            nc.sync.dma_start(
                t10[0:118, 0 : 2 * XTW], x[0:118, ccw : ccw + 2 * XTW]
            )
            nc.sync.dma_start(
                xts[(1, 118)][0:128, 0 : 2 * XTW],
                x[118:246, ccw : ccw + 2 * XTW],
            )
            nc.sync.dma_start(
                t10[0:118, 2 * XTW : cw], x[0:118, ccw + 2 * XTW : 2 * ccw]
            )
            nc.sync.dma_start(
                xts[(1, 118)][0:128, 2 * XTW : cw],
                x[118:246, ccw + 2 * XTW : 2 * ccw],
            )

            o0s, o1s = [], []
            for b in range(BL):
                o0s.append(ob.tile([96, T], F32, name=f"o0_{b}", tag=f"o0_{b}"))
                o1s.append(ob.tile([32, T], F32, name=f"o1_{b}", tag=f"o1_{b}"))

            def mm_block(bi, c, b, psts):
                c0, K, wc, M, pi = BLOCKS[bi]
                if bi == 2:
                    xt = xts[(c, 246)]
                    nc.tensor.matmul(
                        psts[b][1][:, :],
                        wblk2[0:NF, 0:32],
                        xt[0:NF, b * XTW + 2 : b * XTW + 2 + CHUNK],
                        start=True,
                        stop=False,
                    )
                    return
                xt = xts[(c, c0)]
                for j in range(5):
                    wap = wtap[0:K, j * TAP_STRIDE + wc : j * TAP_STRIDE + wc + M]
                    nc.tensor.matmul(
                        psts[b][pi][:, :],
                        wap,
                        xt[0:K, b * XTW + 2 + j : b * XTW + 2 + j + CHUNK],
                        start=(j == 0 and bi != 1),
                        stop=False,
                    )

            def epilogue(c, b, psts):
                corr_off = 0 if c == 0 else CORR_STRIDE
                pcol = 0 if c == 0 else CHUNK - 2
                for bi, (c0, K, wc, M, pi) in enumerate(BLOCKS):
                    xkey = 246 if bi == 2 else c0
                    nc.tensor.matmul(
                        psts[b][pi][:, pcol : pcol + 2],
                        wcor[0:K, corr_off + wc : corr_off + wc + M],
                        xts[(c, xkey)][0:K, b * XTW : b * XTW + 2],
                        start=False,
                        stop=(bi != 1),
                        skip_group_check=True,
                    )
                nc.vector.tensor_copy(
                    o0s[b][:, c * CHUNK : (c + 1) * CHUNK], psts[b][0][:, :]
                )
                nc.scalar.mul(
                    o1s[b][:, c * CHUNK : (c + 1) * CHUNK], psts[b][1][:, :], 1.0
                )
                cs = slice(c * CHUNK, (c + 1) * CHUNK)
                y0_eng = nc.sync if (c == NCHUNK - 1 and b == BL - 1) else nc.gpsimd
                y0_eng.dma_start(y[b * 128 : b * 128 + 96, cs], o0s[b][:, cs])
                nc.scalar.dma_start(
                    y[b * 128 + 96 : b * 128 + 128, cs], o1s[b][:, cs]
                )

            def psum_tiles(c):
                p0s = [
                    ps.tile([96, CHUNK], F32, name=f"p0_{c}_{b}", tag=f"p0_{b}")
                    for b in range(BL)
                ]
                p1s = [
                    ps.tile([32, CHUNK], F32, name=f"p1_{c}_{b}", tag=f"p1_{b}")
                    for b in range(BL)
                ]
                return [(p0s[b], p1s[b]) for b in range(BL)]

            # chunk 0: block-major (matmuls start as soon as each block's
            # DMA lands); each batch's epilogue follows its last block
            psts0 = psum_tiles(0)
            for k in range(6):
                nc.tensor.matmul(
                    psts0[0][0][:, 0:128],
                    scratch[0:128, 0:96],
                    scratch[0:128, 128:256],
                    start=True,
                    stop=(k == 5),
                    skip_group_check=True,
                )
            for bi in (2, 0):
                for b in range(BL):
                    mm_block(bi, 0, b, psts0)
            for b in range(BL):
                mm_block(1, 0, b, psts0)
                epilogue(0, b, psts0)
            # chunk 1: block-major as well (the chunk-1 x transfers land
            # progressively; block-major consumes them in arrival order),
            # with each batch's epilogue right after its last block
            psts1 = psum_tiles(1)
            for bi in (2, 0):
                for b in range(BL):
                    mm_block(bi, 1, b, psts1)
            for b in range(BL):
                mm_block(1, 1, b, psts1)
                epilogue(1, b, psts1)
    _fix_multiwaits(nc)
    return nc


_NC_CACHE = None


def _get_nc():
    global _NC_CACHE
    if _NC_CACHE is None:
        _NC_CACHE = _build_nc()
    return _NC_CACHE


def _round_fp32r(a):
    """Round float32 to fp32r (12 explicit mantissa bits, round-to-nearest;
    matches the PE's reduced-precision operand format)."""
    bits = np.ascontiguousarray(a, dtype=np.float32).view(np.uint32)
    low = bits & np.uint32(0xFFF)
    base = bits & np.uint32(0xFFFFF000)
    up = (low > 0x800) | (
        (low == 0x800) & (((bits >> np.uint32(12)) & np.uint32(1)) == 1)
    )
    return (base + (up.astype(np.uint32) << np.uint32(12))).view(np.float32)


def _prep_inputs(x, w1, b1, w2, b2):
    x = np.asarray(x)
    WTAP, WCOR, WBLK2, bias_field = _build_weights(w1, b1, w2, b2)
    xd = x[:, :, :, 0]  # [B, F, T]
    # [B, FR, NCHUNK, XTW] widened per-chunk layout (see XTW comment).
    # Rows 0..245: channels 0..245 (blocks 0 and 1). Rows 246..300: block-2
    # tap-folded rows — row 246 + k*11 + ch holds channel (246+ch) shifted by
    # tap j=FOLD_JORDER[k]: col m = x[ch, c*CHUNK + m + j - 4].
    FR = 246 + NF
    xw = np.zeros((B, FR, NCHUNK, XTW), dtype=np.float32)
    for c in range(NCHUNK):
        t_lo = c * CHUNK - 2  # t' at col 2
        lo = max(t_lo, 0)
        hi = min(c * CHUNK + CHUNK + 2, T)
        xw[:, 0:246, c, 2 + (lo - t_lo) : 2 + (hi - t_lo)] = xd[:, 0:246, lo:hi]
        for k, j in enumerate(FOLD_JORDER):
            rows = slice(246 + k * 11, 246 + (k + 1) * 11)
            f_lo = c * CHUNK + j - 4  # value at col 0
            lo2 = max(f_lo, 0)
            hi2 = min(f_lo + XTW, T)
            if hi2 > lo2:
                xw[:, rows, c, lo2 - f_lo : hi2 - f_lo] = xd[:, 246:257, lo2:hi2]
    xw[:, 0:246, 0, 0] = xd[:, 0:246, 0]  # corrective staging [x(0), 0]
    xw[:, 0:246, 1, 1] = xd[:, 0:246, T - 1]  # corrective staging [0, x(T-1)]
    # block-2 corrective staging lives on the folded j=2 rows (246..256).
    # The folded data fill can reach cols 0/1 (f_lo >= 0 for chunk 1's
    # j>=2 rows) — clear them first; only rows 246..256 are ever read there.
    xw[:, 246:, :, 0:2] = 0.0
    xw[:, 246:257, 0, 0] = xd[:, 246:257, 0]
    xw[:, 246:257, 1, 1] = xd[:, 246:257, T - 1]
    # device layout per core: [FR, NCHUNK * BL * XTW], col = ((c*BL)+b)*XTW + s
    xs = (
        xw.reshape(N_CORES, BL, FR, NCHUNK, XTW)
        .transpose(0, 2, 3, 1, 4)
        .reshape(N_CORES, FR, NCHUNK * BL * XTW)
    )
    xs = _round_fp32r(np.ascontiguousarray(xs))
    WTAP = _round_fp32r(WTAP)
    WCOR = _round_fp32r(WCOR)
    WBLK2 = _round_fp32r(WBLK2)
    in_maps = [
        {"x": xs[i], "wtap": WTAP, "wcor": WCOR, "wblk2": WBLK2}
        for i in range(N_CORES)
    ]
    return in_maps, bias_field


def kernel(x, w1, b1, w2, b2):
    in_maps, bias_field = _prep_inputs(x, w1, b1, w2, b2)
    res = run_bass_kernel_spmd(_get_nc(), in_maps, core_ids=list(range(N_CORES)))
    out = np.empty((B, 128, T), dtype=np.float64)
    for i in range(N_CORES):
        out[i * BL : (i + 1) * BL] = (
            res.results[i]["y"].reshape(BL, 128, T).astype(np.float64)
        )
    out += bias_field[None, :, :]
    return out.astype(np.float32).reshape(B, N_GROUPS, 8, T, 1)
